# revision 49
# baseline (speedup 1.0000x reference)
"""DeepseekV3 MLA attention on 8 Trainium2 NeuronCores (Bass/Tile).

Sharding: token-parallel. Core c handles batch b = c//4 and 512 query tokens
(4 chunks of 128, zig-zag {a, 7-a, 8+a, 15-a} with a = c%4, ordered by
descending causal-prefix length so every core runs the identical program).
Each core computes the full KV path for its batch (replicated within the
4-core batch group), attention for all 16 heads over its queries, and the
full o-projection for its tokens.  No collectives; host unsharding is pure
concatenation/transposition.

Layouts are transposed (feature-on-partition) end to end; weights arrive
host-pre-transposed; RoPE runs in transposed layout using partition-shifted
single-input copies; softmax is unstabilized exp (scores are ~N(0,1) after
the 1/sqrt(192) scaling, bounded far below f32 overflow); causal masking is
data-driven (per-slot mask tiles multiply probabilities after exp).

Attention-value products are P-stationary: out[q, v] += P_tile.T @ [V | 1]
accumulated in one PSUM bank per 128-query chunk, with the ones column
collecting the softmax denominator; the normalized [q, v] tile is transposed
back to [v, q] on the PE for the o-projection.  Scores for kt+1/kt+2 are
emitted before AV(kt) (the PE is in-order, so AV would otherwise
head-of-line block on the exp).  Bulk weight prefetches are issued from the
Act engine's DMA queue at staged program points so they cannot starve the
latency-critical sync-queue loads (the gpsimd queue runs ahead of program
order); phase pools are ordered so each pool-close barrier (an all-engine
sync) has a cheap tail.
"""

import numpy as np

import concourse.bass as bass
import concourse.mybir as mybir
import concourse.tile as tile
from concourse import bacc
from concourse.masks import make_identity
from concourse.bass_utils import run_bass_kernel_spmd

# Problem constants (hardcoded per contract).
B, S, D = 2, 2048, 2048
H = 16
LQ = 1536           # q low-rank dim
LKV = 512           # kv latent dim
ROPE = 64
NOPE = 128
VDIM = 128
QKD = NOPE + ROPE   # 192
EPS = 1e-6
SCALING = QKD ** -0.5
NT = 512            # query tokens per core
NCORES = 8

F32 = mybir.dt.float32
F32R = mybir.dt.float32r
BF16 = mybir.dt.bfloat16
AF = mybir.ActivationFunctionType
ALU = mybir.AluOpType
AX = mybir.AxisListType


def _blocks_for(a: int) -> list[int]:
    """Query chunk (of 128 tokens) handled by col-block t, t=0..3.

    Block t has causal key prefix covering key groups kg=0..3-t (512 keys
    each); chunk ids are chosen so prefixes are compatible: J_t in
    [12-4t, 15-4t].
    """
    return [15 - a, 8 + a, 7 - a, a]


def build(nheads: int = H, mm_dt=BF16, p_dt=BF16):
    """Build the SPMD Bass program."""
    HP = nheads
    NPAIR = (HP + 1) // 2
    HG = (HP + 3) // 4
    GSZ = min(4, HP)
    NLQ = LQ // 128      # 12
    NLKV = LKV // 128    # 4

    nc = bacc.Bacc("TRN2", target_bir_lowering=False, debug=False,
                   num_devices=NCORES)

    def din(name, shape, dt=mm_dt):
        return nc.dram_tensor(name, list(shape), dt, kind="ExternalInput")

    hidT = din("hidT", [D, S])
    hidTq = din("hidTq", [D, NT])
    qaWt = din("qaWt", [D, LQ])
    qbWp = din("qbWp", [LQ, HP * NOPE])
    qbWr = din("qbWr", [LQ, HP * ROPE])
    kvaWt = din("kvaWt", [D, LKV + ROPE])
    kvbWk = din("kvbWk", [LKV, HP * NOPE])
    kvbWv = din("kvbWv", [LKV, HP * VDIM])
    oWt = din("oWt", [HP * VDIM, D])
    cosq2 = din("cosq2", [2 * ROPE, NT])
    sinq2 = din("sinq2", [2 * ROPE, NT])        # first half negated
    cosk = din("cosk", [ROPE, S])
    sink = din("sink", [ROPE, S])               # first half negated
    masks = din("masks", [16, 128, 128])
    outT = nc.dram_tensor("outT", [D, NT], F32, kind="ExternalOutput")

    with tile.TileContext(nc) as tc:
        with (
            tc.tile_pool(name="psum", bufs=1, space="PSUM") as pp,
            tc.tile_pool(name="peC", bufs=1) as peC,
            tc.tile_pool(name="small", bufs=1) as sm,
        ):
            # PSUM bank budget (8 banks): score x2, kv x2, av0..av3 x1.
            def ps_score():
                return pp.tile([128, 512], F32, tag="score", bufs=2,
                               name="ps_score")

            def ps_av(i):
                return pp.tile([128, 512], F32, tag=f"av{i}", bufs=1,
                               name=f"ps_av{i}")

            def ps_kv():
                return pp.tile([128, 512], F32, tag="kv", bufs=2,
                               name="ps_kv")

            def ps_tp(dt):
                # transpose scratch: aliases the kv banks (same 2KB/partition)
                n = 1024 if dt != F32 else 512
                return pp.tile([128, n], dt, tag="kv", bufs=2, name="ps_tp")

            # identities, eps
            ident_f = sm.tile([128, 128], F32)
            make_identity(nc, ident_f[:])
            ident_r = sm.tile([128, 128], mm_dt)
            nc.vector.tensor_copy(ident_r[:], ident_f[:])
            epsb = sm.tile([128, 1], F32)
            nc.vector.memset(epsb[:], EPS)

            # Long-lived (through phase C) tensors.
            qp = peC.tile([128, HP, NT], mm_dt)      # q_pass^T per head
            vw0 = peC.tile([128, NLKV, GSZ * VDIM], mm_dt)   # C hg=0 V wts
            kwg0 = peC.tile([128, NLKV, GSZ * NOPE], mm_dt)  # C hg=0 K wts
            qr = peC.tile([128, NPAIR, NT], mm_dt)   # roped q_rot^T head pairs
            krT = peC.tile([128, S], mm_dt)          # roped k_rot^T (dup halves)
            cT = peC.tile([128, NLKV, S], mm_dt)     # c-tilde^T
            at = peC.tile([128, HP, NT], mm_dt)      # attn out, head-major
            # V tiles per key-tile: [V | 1] with the ones column collecting
            # the softmax denominator.  Manually double-buffered across hgs.
            v4b = peC.tile([128, 2, 16, GSZ, 129], mm_dt)
            for vb in range(2):
                nc.vector.memset(v4b[:, vb, :, 0:GSZ, 128:129], 1.0)

            # ============ Phase A-q: q_a + rmsnorm + transpose ==========
            with tc.tile_pool(name="phaq", bufs=1) as ph:
                qT = ph.tile([128, NLQ, NT], mm_dt)
                # next-phase tiles, DMA-prefetched during A-q compute
                kvw = ph.tile([128, 16, LKV + ROPE], mm_dt)
                ck = ph.tile([ROPE, S], mm_dt, tag="ck")
                sk = ph.tile([ROPE, S], mm_dt, tag="sk")
                hid0 = ph.tile([128, 8, 512], mm_dt, tag="hid0")
                w0 = ph.tile([128, NLQ, 2 * NOPE], mm_dt, tag="qbw0")
                wr_pre = [ph.tile([128, NLQ, 2 * ROPE], mm_dt,
                                  tag=f"qrw0{i}", name=f"wr_pre{i}")
                          for i in range(2)]
                cq = ph.tile([2 * ROPE, NT], mm_dt, tag="cq")
                sq2 = ph.tile([2 * ROPE, NT], mm_dt, tag="sq2")
                with tc.tile_pool(name="phaq2", bufs=1) as ph2:
                    hq = ph2.tile([128, 16, NT], mm_dt)
                    qnat = ph2.tile([128, 4, LQ], mm_dt)
                    ssq = ph2.tile([128, 4], F32)  # per-tq sum of squares

                    # DMA issue order: hq chunk 0 and the first two qaw tiles
                    # first (sync queue) so matmuls start ~3us in; bulk
                    # prefetches ride the idle gpsimd queue.
                    def dma_hq(hc):
                        nc.sync.dma_start(
                            hq[:, 4 * hc:4 * (hc + 1), :],
                            hidTq[hc * 512:(hc + 1) * 512, :].rearrange(
                                "(dk p) t -> p dk t", p=128))

                    def dma_qaw(g, dk2, bufs=3):
                        qaw = ph2.tile([128, 2, 512], mm_dt, tag="qaw",
                                       bufs=bufs)
                        nc.sync.dma_start(
                            qaw[:],
                            qaWt[dk2 * 256:(dk2 + 1) * 256,
                                 g * 512:(g + 1) * 512].rearrange(
                                "(two p) c -> p two c", p=128))
                        return qaw

                    dma_hq(0)
                    qaw_pre = {(0, 0): dma_qaw(0, 0), (0, 1): dma_qaw(0, 1)}
                    dma_hq(1)
                    qaw_pre[(0, 2)] = dma_qaw(0, 2)
                    qaw_pre[(0, 3)] = dma_qaw(0, 3)
                    dma_hq(2)
                    dma_hq(3)
                    for g in range(3):
                        if g % 2 == 0:
                            accs = [ps_av(0), ps_av(1), ps_av(2), ps_av(3)]
                        else:
                            accs = [ps_score(), ps_score(), ps_kv(), ps_kv()]
                        for dk2 in range(8):
                            qaw = qaw_pre.pop((g, dk2), None)
                            if qaw is None:
                                qaw = dma_qaw(g, dk2)
                            for ti in range(2):
                                dk = dk2 * 2 + ti
                                for tq in range(4):
                                    nc.tensor.matmul(
                                        accs[tq][:],
                                        hq[:, dk, tq * 128:(tq + 1) * 128],
                                        qaw[:, ti, :],
                                        start=(dk == 0), stop=(dk == 15))
                        for tq in range(4):
                            sq = ph2.tile([128, 512], mm_dt, tag="sq", bufs=2)
                            ps = sm.tile([128, 1], F32, tag="ss", bufs=4)
                            nc.scalar.activation(out=sq[:], in_=accs[tq][:],
                                                 func=AF.Square,
                                                 accum_out=ps[:])
                            if g == 0:
                                nc.vector.tensor_copy(ssq[:, tq:tq + 1],
                                                      ps[:])
                            else:
                                nc.vector.tensor_add(ssq[:, tq:tq + 1],
                                                     ssq[:, tq:tq + 1], ps[:])
                            nc.vector.tensor_copy(
                                qnat[:, tq, g * 512:(g + 1) * 512], accs[tq][:])
                        if g == 1:
                            # A-kv prefetches, triggered from the Act queue so
                            # the transfers start only once Act reaches this
                            # point (the gpsimd queue runs ahead and would
                            # steal A-q's DMA bandwidth).
                            for kc in range(4):
                                nc.scalar.dma_start(
                                    kvw[:, 4 * kc:4 * (kc + 1), :],
                                    kvaWt[kc * 512:(kc + 1) * 512, :]
                                    .rearrange("(dk p) c -> p dk c", p=128))
                            for kc in range(2):
                                nc.scalar.dma_start(
                                    hid0[:, 4 * kc:4 * (kc + 1), :],
                                    hidT[kc * 512:(kc + 1) * 512, 0:512]
                                    .rearrange("(dk p) t -> p dk t", p=128))
                    # prefetch rope tables + B-q first weight tiles
                    nc.scalar.dma_start(ck[:], cosk[:])
                    nc.scalar.dma_start(sk[:], sink[:])
                    nc.scalar.dma_start(
                        w0[:],
                        qbWp[:, 0:2 * NOPE].rearrange("(lk p) c -> p lk c",
                                                      p=128))
                    nc.scalar.dma_start(cq[:], cosq2[:])
                    nc.scalar.dma_start(sq2[:], sinq2[:])
                    for i in range(2):
                        nc.scalar.dma_start(
                            wr_pre[i][:],
                            qbWr[:, i * 2 * ROPE:(i + 1) * 2 * ROPE]
                            .rearrange("(lk p) c -> p lk c", p=128))
                    # rsqrt + scale + transpose
                    nc.scalar.activation(out=ssq[:], in_=ssq[:], func=AF.Sqrt,
                                         bias=epsb[:], scale=1.0 / LQ)
                    nc.vector.reciprocal(out=ssq[:], in_=ssq[:])
                    for tq in range(4):
                        if tq % 2 == 0:
                            nc.vector.tensor_scalar(
                                out=qnat[:, tq, :], in0=qnat[:, tq, :],
                                scalar1=ssq[:, tq:tq + 1], scalar2=None,
                                op0=ALU.mult)
                        else:
                            nc.scalar.activation(
                                out=qnat[:, tq, :], in_=qnat[:, tq, :],
                                func=AF.Copy, scale=ssq[:, tq:tq + 1])
                    for lk in range(NLQ):
                        tp = ps_tp(mm_dt)
                        for tq in range(4):
                            nc.tensor.transpose(
                                tp[:, tq * 128:(tq + 1) * 128],
                                qnat[:, tq, lk * 128:(lk + 1) * 128],
                                ident_r[:])
                        if lk % 2 == 0:
                            nc.scalar.copy(qT[:, lk, :], tp[:, 0:512])
                        else:
                            nc.vector.tensor_copy(qT[:, lk, :], tp[:, 0:512])

                # ==== Phase A-kv: kv_a + rmsnorm + k-rope + transpose ===
                # Two-pass per 512-token chunk (tk 0/1 then tk 2/3) so the
                # norm/rope/transpose chain of one pass overlaps the next
                # pass's matmuls instead of head-of-line blocking the PE.
                with tc.tile_pool(name="phakv", bufs=1) as ph4:
                    hid_cache = {}

                    def get_hid(q4, half):
                        if q4 == 0 and half == 0:
                            return hid0
                        key = (q4, half)
                        if key not in hid_cache:
                            hid = ph4.tile([128, 8, 512], mm_dt,
                                           tag="hid", bufs=3)
                            nc.sync.dma_start(
                                hid[:],
                                hidT[half * 1024:(half + 1) * 1024,
                                     q4 * 512:(q4 + 1) * 512].rearrange(
                                    "(dk p) t -> p dk t", p=128))
                            hid_cache[key] = hid
                        return hid_cache[key]

                    def norm2(q4, tks, accs2):
                        for tk, acc in zip(tks, accs2):
                            tt = q4 * 4 + tk
                            sq = ph4.tile([128, LKV], mm_dt, tag="sqkv",
                                          bufs=2)
                            ss = sm.tile([128, 1], F32, tag="ss", bufs=4)
                            nc.scalar.activation(out=sq[:], in_=acc[:],
                                                 func=AF.Square,
                                                 accum_out=ss[:])
                            nc.scalar.activation(out=ss[:], in_=ss[:],
                                                 func=AF.Sqrt, bias=epsb[:],
                                                 scale=1.0 / LKV)
                            nc.vector.reciprocal(out=ss[:], in_=ss[:])
                            cnt = ph4.tile([128, LKV], mm_dt, tag="cnt",
                                           bufs=3)
                            nc.vector.tensor_scalar(
                                out=cnt[:], in0=acc[:],
                                scalar1=ss[:], scalar2=None, op0=ALU.mult)
                            tp = ps_tp(mm_dt)
                            for lk in range(NLKV):
                                nc.tensor.transpose(
                                    tp[:, lk * 128:(lk + 1) * 128],
                                    cnt[:, lk * 128:(lk + 1) * 128],
                                    ident_r[:])
                            nc.scalar.copy(
                                cT[:, :, tt * 128:(tt + 1) * 128],
                                tp[:, 0:512].rearrange(
                                    "p (lk c) -> p lk c", c=128))

                    def rope_k(q4, rot):
                        kR = ph4.tile([ROPE, 512], F32, tag="kR", bufs=2)
                        kS = ph4.tile([ROPE, 512], F32, tag="kS", bufs=2)
                        nc.scalar.copy(kR[:], rot[0:ROPE, :])
                        nc.scalar.copy(kS[0:32, :], rot[32:64, :])
                        nc.scalar.copy(kS[32:64, :], rot[0:32, :])
                        cs = ck[:, q4 * 512:(q4 + 1) * 512]
                        sn = sk[:, q4 * 512:(q4 + 1) * 512]
                        nc.vector.tensor_mul(kR[:], kR[:], cs)
                        nc.vector.tensor_mul(kS[:], kS[:], sn)
                        nc.vector.tensor_add(
                            krT[0:ROPE, q4 * 512:(q4 + 1) * 512],
                            kR[:], kS[:])
                        nc.vector.tensor_add(
                            krT[ROPE:2 * ROPE, q4 * 512:(q4 + 1) * 512],
                            kR[:], kS[:])

                    pending = None
                    for q4 in range(4):
                        acc01 = [ps_av(0), ps_av(1)]
                        rot = ps_kv()
                        for half in range(2):
                            hid = get_hid(q4, half)
                            for dk8 in range(8):
                                dk = half * 8 + dk8
                                for i, tk in enumerate((0, 1)):
                                    nc.tensor.matmul(
                                        acc01[i][:],
                                        hid[:, dk8, tk * 128:(tk + 1) * 128],
                                        kvw[:, dk, 0:LKV],
                                        start=(dk == 0), stop=(dk == 15))
                                nc.tensor.matmul(rot[0:ROPE, :],
                                                 kvw[:, dk, LKV:LKV + ROPE],
                                                 hid[:, dk8, :],
                                                 start=(dk == 0),
                                                 stop=(dk == 15))
                            if half == 0 and pending is not None:
                                # previous chunk's tk2/3 norm: its DVE chain
                                # ran during the half-0 matmuls above.
                                pending()
                                pending = None
                        acc23 = [ps_av(2), ps_av(3)]
                        for half in range(2):
                            hid = get_hid(q4, half)
                            for dk8 in range(8):
                                dk = half * 8 + dk8
                                for i, tk in enumerate((2, 3)):
                                    nc.tensor.matmul(
                                        acc23[i][:],
                                        hid[:, dk8, tk * 128:(tk + 1) * 128],
                                        kvw[:, dk, 0:LKV],
                                        start=(dk == 0), stop=(dk == 15))
                        rope_k(q4, rot)
                        norm2(q4, (0, 1), acc01)

                        def pending(q4=q4, acc23=acc23):
                            norm2(q4, (2, 3), acc23)
                    pending()

                # ============ Phase B-q: q_b + q-rope (uses qT) =========
                # qr first so its multi-op rope tail drains under the qp
                # matmuls; the hg=0 V tiles are computed at the end of this
                # pool so the pool-close barrier costs no PE time.
                with tc.tile_pool(name="phbq", bufs=1) as ph3:
                    # prefetch phase-C first head-group weights early
                    nc.gpsimd.dma_start(
                        vw0[:],
                        kvbWv[:, 0:GSZ * VDIM].rearrange(
                            "(lk p) c -> p lk c", p=128))
                    nc.gpsimd.dma_start(
                        kwg0[:],
                        kvbWk[:, 0:GSZ * NOPE].rearrange(
                            "(lk p) c -> p lk c", p=128))
                    for p in range(NPAIR):
                        npr = min(2 * ROPE, HP * ROPE - p * 2 * ROPE)
                        if p < 2:
                            w = wr_pre[p]
                        else:
                            w = ph3.tile([128, NLQ, 2 * ROPE], mm_dt,
                                         tag="qrw", bufs=3)
                            nc.sync.dma_start(
                                w[:, :, 0:npr],
                                qbWr[:, p * 2 * ROPE:p * 2 * ROPE
                                     + npr].rearrange(
                                    "(lk p) c -> p lk c", p=128))
                        acc = ps_av(p % 4)
                        for lk in range(NLQ):
                            nc.tensor.matmul(acc[0:npr, :], w[:, lk, 0:npr],
                                             qT[:, lk, :],
                                             start=(lk == 0),
                                             stop=(lk == NLQ - 1))
                        qR = ph3.tile([128, NT], F32, tag="qR", bufs=2)
                        qS = ph3.tile([128, NT], F32, tag="qS", bufs=2)
                        for hh in range(npr // ROPE):
                            r0 = hh * ROPE
                            nc.scalar.copy(qS[r0:r0 + 32, :],
                                           acc[r0 + 32:r0 + 64, :])
                            nc.scalar.copy(qS[r0 + 32:r0 + 64, :],
                                           acc[r0:r0 + 32, :])
                        nc.vector.tensor_mul(qR[0:npr, :], acc[0:npr, :],
                                             cq[0:npr, :])
                        nc.vector.tensor_mul(qS[0:npr, :], qS[0:npr, :],
                                             sq2[0:npr, :])
                        nc.vector.tensor_add(qr[0:npr, p, :], qR[0:npr, :],
                                             qS[0:npr, :])
                    for h2 in range((HP + 1) // 2):
                        nh = min(2, HP - h2 * 2)
                        if h2 == 0:
                            w = w0
                        else:
                            w = ph3.tile([128, NLQ, 2 * NOPE], mm_dt,
                                         tag="qbw", bufs=3)
                            nc.sync.dma_start(
                                w[:, :, 0:nh * NOPE],
                                qbWp[:, h2 * 2 * NOPE:
                                     (h2 * 2 + nh) * NOPE].rearrange(
                                    "(lk p) c -> p lk c", p=128))
                        for hh in range(nh):
                            h = h2 * 2 + hh
                            acc = ps_score() if h % 2 == 0 else ps_kv()
                            for lk in range(NLQ):
                                nc.tensor.matmul(
                                    acc[:],
                                    w[:, lk, hh * NOPE:(hh + 1) * NOPE],
                                    qT[:, lk, :],
                                    start=(lk == 0), stop=(lk == NLQ - 1))
                            if h % 2 == 0:
                                nc.scalar.copy(qp[:, h, :], acc[:])
                            else:
                                nc.vector.tensor_copy(qp[:, h, :], acc[:])
                    # hg=0 V tiles (reads only persistent tensors)
                    for tt in range(16):
                        acc = ps_kv()
                        for lk in range(NLKV):
                            nc.tensor.matmul(
                                acc[:, 0:GSZ * VDIM],
                                cT[:, lk, tt * 128:(tt + 1) * 128],
                                vw0[:, lk, 0:GSZ * VDIM],
                                start=(lk == 0), stop=(lk == NLKV - 1))
                        nc.scalar.copy(
                            v4b[:, 0, tt, 0:GSZ, 0:128],
                            acc[:, 0:GSZ * VDIM].rearrange(
                                "p (g s) -> p g s", s=128))

            # ============ Phase C: attention ============================
            # Transposed scores: S^T[k, q] = K @ Q^T per 128-key tile.  AV is
            # P-stationary: out[q, v] += P-tile.T @ [V | 1] accumulated in one
            # PSUM bank per 128-query chunk; column 128 collects the softmax
            # denominator for free.  The normalized [q, v] tile is transposed
            # back to [v, q] on the PE for the o-projection.
            with tc.tile_pool(name="phc", bufs=1) as ph:
                msk = ph.tile([128, 16, 128], mm_dt)
                nc.gpsimd.dma_start(msk[:], masks.rearrange("t p c -> p t c"))
                ow0 = ph.tile([128, HP, 512], mm_dt, tag="ow0")
                drain = None

                def fetch_cw(hg):
                    """Load head-group hg's K/V weights (hg=0 prefetched in
                    phase B)."""
                    h0 = hg * GSZ
                    gs = min(GSZ, HP - h0)
                    vw = ph.tile([128, NLKV, GSZ * VDIM], mm_dt,
                                 tag="vw", bufs=2)
                    nc.gpsimd.dma_start(
                        vw[:, :, 0:gs * VDIM],
                        kvbWv[:, h0 * VDIM:(h0 + gs) * VDIM].rearrange(
                            "(lk p) c -> p lk c", p=128))
                    kwg = ph.tile([128, NLKV, GSZ * NOPE], mm_dt,
                                  tag="kwg", bufs=2)
                    nc.gpsimd.dma_start(
                        kwg[:, :, 0:gs * NOPE],
                        kvbWk[:, h0 * NOPE:(h0 + gs) * NOPE].rearrange(
                            "(lk p) c -> p lk c", p=128))
                    return vw, kwg

                next_w = (vw0, kwg0)
                for hg in range(HG):
                    if hg == 2:
                        nc.gpsimd.dma_start(
                            ow0[:],
                            oWt[:, 0:512].rearrange("(hk p) c -> p hk c",
                                                    p=128))
                    h0 = hg * GSZ
                    gs = min(GSZ, HP - h0)
                    vw, kwg = next_w
                    if hg + 1 < HG:
                        next_w = fetch_cw(hg + 1)
                    vb = hg % 2
                    if hg > 0:
                        for tt in range(16):
                            acc = ps_kv()
                            for lk in range(NLKV):
                                nc.tensor.matmul(
                                    acc[:, 0:gs * VDIM],
                                    cT[:, lk, tt * 128:(tt + 1) * 128],
                                    vw[:, lk, 0:gs * VDIM],
                                    start=(lk == 0), stop=(lk == NLKV - 1))
                            nc.scalar.copy(
                                v4b[:, vb, tt, 0:gs, 0:128],
                                acc[:, 0:gs * VDIM].rearrange(
                                    "p (g s) -> p g s", s=128))
                    for hh in range(gs):
                        h = h0 + hh
                        kh = ph.tile([128, S], mm_dt, tag="kh", bufs=2)
                        for kg4 in range(4):
                            acc = ps_kv()
                            for lk in range(NLKV):
                                nc.tensor.matmul(
                                    acc[:],
                                    kwg[:, lk, hh * NOPE:(hh + 1) * NOPE],
                                    cT[:, lk, kg4 * 512:(kg4 + 1) * 512],
                                    start=(lk == 0), stop=(lk == NLKV - 1))
                            nc.vector.tensor_copy(
                                kh[:, kg4 * 512:(kg4 + 1) * 512], acc[:])
                        # drain the PREVIOUS head's AV accumulators here so
                        # its DVE work overlaps this head's K matmuls.
                        if drain is not None:
                            drain()
                            drain = None
                        r0 = (h % 2) * ROPE
                        avacc = [ps_av(0), ps_av(1), ps_av(2), ps_av(3)]
                        atq = ph.tile([128, 512], mm_dt, tag="atq", bufs=2)

                        # score-tile allocator: the two score banks, plus the
                        # av3/av2 banks after their accumulations drain at
                        # kt=3/7 — gives the small-N tail a depth-3 pipeline.
                        # (Borrow points are chosen so each borrowed write is
                        # EMITTED after that bank's drain in program order.)
                        def sc_tile(kt):
                            if kt in (8, 10):
                                return ps_av(3)
                            if kt in (12, 14):
                                return ps_av(2)
                            return ps_score()

                        def scores(kt, kh=kh, h=h, r0=r0):
                            nb = 4 - kt // 4
                            N = nb * 128
                            sc = sc_tile(kt)
                            nc.tensor.matmul(
                                sc[:, 0:N],
                                kh[:, kt * 128:(kt + 1) * 128],
                                qp[:, h, 0:N],
                                start=True, stop=False)
                            nc.tensor.matmul(
                                sc[:, 0:N],
                                krT[r0:r0 + ROPE, kt * 128:(kt + 1) * 128],
                                qr[r0:r0 + ROPE, h // 2, 0:N],
                                start=False, stop=True)
                            pt_ = ph.tile([128, 512], p_dt, tag="P", bufs=6)
                            nc.scalar.activation(out=pt_[:, 0:N],
                                                 in_=sc[:, 0:N], func=AF.Exp,
                                                 scale=SCALING)
                            nc.vector.tensor_mul(pt_[:, N - 128:N],
                                                 pt_[:, N - 128:N],
                                                 msk[:, kt, :])
                            return pt_

                        # software pipeline: depth 2 while all four AV
                        # accumulators are live, ramping to depth 4 in the
                        # small-N tail.  Each q-chunk's accumulator is
                        # normalized (recip + scale on DVE) right after its
                        # last AV matmul, freeing its bank for the allocator
                        # above and spreading the drain work.
                        pts = {0: scores(0), 1: scores(1)}
                        emitted = 1
                        for kt in range(16):
                            nb = 4 - kt // 4
                            pt_ = pts.pop(kt)
                            depth = 2 if kt < 5 else 3
                            while emitted + 1 < 16 and emitted - kt < depth:
                                emitted += 1
                                pts[emitted] = scores(emitted)
                            for t in range(nb):
                                stop = kt == (4 - t) * 4 - 1
                                nc.tensor.matmul(
                                    avacc[t][:, 0:129],
                                    pt_[:, t * 128:(t + 1) * 128],
                                    v4b[:, vb, kt, hh, 0:129],
                                    start=(kt == 0), stop=stop,
                                    skip_group_check=True)
                                if stop:
                                    rd = sm.tile([128, 1], F32, tag="rd",
                                                 bufs=4)
                                    nc.vector.reciprocal(
                                        out=rd[:], in_=avacc[t][:, 128:129])
                                    nc.vector.tensor_scalar(
                                        out=atq[:, t * 128:(t + 1) * 128],
                                        in0=avacc[t][:, 0:128],
                                        scalar1=rd[:], scalar2=None,
                                        op0=ALU.mult)

                        def drain(atq=atq, h=h):
                            tp = ps_tp(mm_dt)
                            for t in range(4):
                                nc.tensor.transpose(
                                    tp[:, t * 128:(t + 1) * 128],
                                    atq[:, t * 128:(t + 1) * 128],
                                    ident_r[:])
                            nc.scalar.copy(at[:, h, :], tp[:, 0:512])
                if drain is not None:
                    drain()
                    drain = None

                # ============ Phase D: o-projection =====================
                for dq in range(4):
                    if dq == 0:
                        ow = ow0
                    else:
                        ow = ph.tile([128, HP, 512], mm_dt, tag="ow", bufs=2)
                        nc.gpsimd.dma_start(
                            ow[:],
                            oWt[:, dq * 512:(dq + 1) * 512].rearrange(
                                "(hk p) c -> p hk c", p=128))
                    accs = [ps_av(0), ps_av(1), ps_av(2), ps_av(3)]
                    for dt in range(4):
                        last = (dq == 3 and dt == 3)
                        if not last:
                            halves = [(accs[dt], 0, 512)]
                        else:
                            # split the final accumulation by token halves so
                            # the first half's copy+store overlaps the second
                            # half's matmuls (trims the end-of-kernel drain)
                            halves = [(accs[dt], 0, 256), (ps_kv(), 256, 512)]
                        for acc, c0, c1 in halves:
                            for hk in range(HP):
                                nc.tensor.matmul(
                                    acc[:, 0:c1 - c0],
                                    ow[:, hk, dt * 128:(dt + 1) * 128],
                                    at[:, hk, c0:c1],
                                    start=(hk == 0), stop=(hk == HP - 1))
                            ot = ph.tile([128, 512], F32, tag="ot", bufs=3)
                            if dt % 2 == 0:
                                nc.scalar.copy(ot[:, c0:c1],
                                               acc[:, 0:c1 - c0])
                            else:
                                nc.vector.tensor_copy(ot[:, c0:c1],
                                                      acc[:, 0:c1 - c0])
                            d0 = dq * 512 + dt * 128
                            nc.sync.dma_start(outT[d0:d0 + 128, c0:c1],
                                              ot[:, c0:c1])

    nc.compile()
    return nc


def prep_inputs(inputs: dict, nheads: int = H) -> list[dict]:
    """Shard + pre-transpose the full inputs into 8 per-core input maps."""
    import ml_dtypes
    bf16 = ml_dtypes.bfloat16
    f32 = np.float32
    hs = np.asarray(inputs["hidden_states"], f32)
    cos = np.asarray(inputs["cos"], f32)
    sin = np.asarray(inputs["sin"], f32)
    qaW = np.asarray(inputs["q_a_W"], f32)
    qanw = np.asarray(inputs["q_a_norm_w"], f32)
    qbW = np.asarray(inputs["q_b_W"], f32)
    kvaW = np.asarray(inputs["kv_a_W"], f32)
    kvanw = np.asarray(inputs["kv_a_norm_w"], f32)
    kvbW = np.asarray(inputs["kv_b_W"], f32)
    oW = np.asarray(inputs["o_W"], f32)

    HP = nheads
    qaWt = np.ascontiguousarray(qaW.T)                      # [D, LQ]
    # fold q_a_norm_w into q_b rows (columns of q_b_W)
    qbWs = qbW[: HP * QKD] * qanw[None, :]                  # [HP*QKD, LQ]
    qb3 = qbWs.reshape(HP, QKD, LQ)
    qbWp = np.ascontiguousarray(
        qb3[:, :NOPE, :].reshape(HP * NOPE, LQ).T)          # [LQ, HP*128]
    qbWr = np.ascontiguousarray(
        qb3[:, NOPE:, :].reshape(HP * ROPE, LQ).T)          # [LQ, HP*64]
    kvaWt = np.ascontiguousarray(kvaW.T)                    # [D, 576]
    kvb3 = (kvbW[: HP * (NOPE + VDIM)] * kvanw[None, :]).reshape(
        HP, NOPE + VDIM, LKV)
    kvbWk = np.ascontiguousarray(
        kvb3[:, :NOPE, :].reshape(HP * NOPE, LKV).T)        # [LKV, HP*128]
    kvbWv = np.ascontiguousarray(
        kvb3[:, NOPE:, :].reshape(HP * VDIM, LKV).T)        # [LKV, HP*128]
    oWt = np.ascontiguousarray(oW[:, : HP * VDIM].T)        # [HP*128, D]

    qaWt = qaWt.astype(bf16)
    qbWp = qbWp.astype(bf16)
    qbWr = qbWr.astype(bf16)
    kvaWt = kvaWt.astype(bf16)
    kvbWk = kvbWk.astype(bf16)
    kvbWv = kvbWv.astype(bf16)
    oWt = oWt.astype(bf16)
    hTb = [np.ascontiguousarray(hs[b].T).astype(bf16) for b in range(B)]

    in_maps = []
    for c in range(NCORES):
        b, a = divmod(c, 4)
        blocks = _blocks_for(a)
        qidx = np.concatenate(
            [np.arange(j * 128, (j + 1) * 128) for j in blocks])
        hT = hTb[b]                                         # [D, S] bf16
        cosT = np.ascontiguousarray(cos[b].T)               # [64, S]
        sinT = np.ascontiguousarray(sin[b].T)
        sinneg = sinT.copy()
        sinneg[:32] = -sinneg[:32]
        cosq = cosT[:, qidx]
        sinq = sinneg[:, qidx]
        hTq = np.ascontiguousarray(hT[:, qidx])             # [D, NT] bf16
        mk = np.zeros((16, 128, 128), f32)
        for kt in range(16):
            t = 3 - kt // 4
            j = blocks[t]
            qpos = np.arange(j * 128, (j + 1) * 128)
            kpos = kt * 128 + np.arange(128)
            mk[kt] = (kpos[:, None] <= qpos[None, :]).astype(f32)
        in_maps.append({
            "hidT": hT,
            "hidTq": hTq,
            "qaWt": qaWt, "qbWp": qbWp, "qbWr": qbWr,
            "kvaWt": kvaWt, "kvbWk": kvbWk, "kvbWv": kvbWv, "oWt": oWt,
            "cosq2": np.ascontiguousarray(
                np.concatenate([cosq, cosq], 0)).astype(bf16),
            "sinq2": np.ascontiguousarray(
                np.concatenate([sinq, sinq], 0)).astype(bf16),
            "cosk": cosT.astype(bf16), "sink": sinneg.astype(bf16),
            "masks": mk.astype(bf16),
        })
    return in_maps


def assemble(results: list[dict]) -> np.ndarray:
    out = np.empty((B, S, D), np.float32)
    for c in range(NCORES):
        b, a = divmod(c, 4)
        blocks = _blocks_for(a)
        oT = results[c]["outT"]                             # [D, 512]
        for t, j in enumerate(blocks):
            out[b, j * 128:(j + 1) * 128, :] = oT[:, t * 128:(t + 1) * 128].T
    return out


_CACHE = {}


def _get_nc(nheads=H):
    key = nheads
    if key not in _CACHE:
        _CACHE[key] = build(nheads)
    return _CACHE[key]


def kernel(**inputs) -> np.ndarray:
    nc = _get_nc()
    in_maps = prep_inputs(inputs)
    res = run_bass_kernel_spmd(nc, in_maps, list(range(NCORES)))
    return assemble(res.results)

